# revision 23
# baseline (speedup 1.0000x reference)
"""Embedding lookup (gather) kernel for Trainium2, 8 NeuronCores.

Problem: out[i] = table[value_tensors[i]] for 212992 indices into a
[1M, 128] f32 table, reshaped to [8192, 26, 128]. (row_offsets is
arange, so the CSR segment-sum is the identity; a host-side fallback
handles the general case.)

Sharding: model-parallel by table row (range partition). The table is
split into 32 range bins of 31250 rows; core c owns bins 4c..4c+3
(125000 rows = 64MB per core). The host routes each lookup index to its
owning bin, each core gathers its rows on-device with the SWDGE
dma_gather instruction (one per bin; int16 local indices < 31250), and
the host scatters the gathered rows back to the original positions
(the "all-to-all" step of HugeCTR's localized embedding, done at
unshard time).

dma_gather layout (probed on HW): indices are int16, wrapped over 16
partitions (element i at [i % 16, i // 16]) and replicated to all 8
Q7-core partition groups; gathered row i lands at dst[i % 128, i // 128].
"""

import math

import numpy as np

VOCAB = 1_000_000
BATCH = 8192
SLOTS = 26
VEC = 128
NCORES = 8
NSUB = 4  # sub-shards (bins) per core; int16 gather idx needs rows <= 32767
RSUB = VOCAB // (NCORES * NSUB)  # 31250 rows per bin
SHARD = RSUB * NSUB  # 125000 rows per core
P = 128

LAST_RUN = None  # BassKernelResults of the most recent device run (for test.py)


def _build_program(N: int):
    """One SPMD program for all 8 cores. N = padded lookups per bin
    (multiple of 128; identical across cores/bins so num_idxs is a
    compile-time constant).

    Per core:
      shard [SHARD, VEC] f32      - this core's 4 bins, concatenated
      idx   [P, NSUB*S] int16     - wrapped local indices, S = N//16
      out   [P, NSUB*C*VEC] f32   - gathered rows, C = N//128
    """
    import concourse.bacc as bacc
    from concourse import mybir
    from concourse.library_config import mlp

    S = N // 16
    C = N // 128
    # Idxs per dma_gather: 768 -> 48 data descs + 1 sem desc per engine ring,
    # safely under the 64-descriptor packet ceiling (1024 -> 65 descs, which
    # is at/over the limit and produced rare device lockups).
    CH = 768

    chunks = []  # (start, size) within a bin, multiples of 128
    o = 0
    while o < N:
        chunks.append((o, min(CH, N - o)))
        o += CH
    nch = len(chunks)

    nc = bacc.Bacc("TRN2", num_swdge_queues=4)
    shard = nc.declare_dram_parameter(
        "shard", [SHARD, VEC], mybir.dt.float32, isOutput=False
    )
    idx = nc.declare_dram_parameter(
        "idx", [P, NSUB * S], mybir.dt.int16, isOutput=False
    )
    out = nc.declare_dram_parameter(
        "out", [P, NSUB * C * VEC], mybir.dt.float32, isOutput=True
    )

    sem_idx = nc.alloc_semaphore()
    sem_g = [
        nc.alloc_semaphore(f"sem_g{s}_{j}") for s in range(NSUB) for j in range(nch)
    ]
    sem_out = nc.alloc_semaphore()

    idx_sb = nc.alloc_sbuf_tensor("idx_sb", [P, NSUB * S], mybir.dt.int16).ap()
    g_bufs = [
        nc.alloc_sbuf_tensor(f"g{s}", [P, C, VEC], mybir.dt.float32).ap()
        for s in range(NSUB)
    ]

    nc.gpsimd.load_library(mlp)
    nc.sync.dma_start(out=idx_sb[:], in_=idx[:, :]).then_inc(sem_idx, 16)
    nc.gpsimd.wait_ge(sem_idx, 16)
    # Hoist num_idxs registers: one MOVE per distinct chunk size instead of
    # one per gather (each MOVE costs ~420ns of Pool sequencer time).
    size_regs = {sz: nc.gpsimd.to_reg(sz) for sz in sorted({sz for _, sz in chunks})}
    for s in range(NSUB):
        for j, (o, sz) in enumerate(chunks):
            # Round-robin over SWDGE queues: each queue_num runs on its own
            # Q7 core pair, parallelizing descriptor generation 4x.
            nc.gpsimd.dma_gather(
                g_bufs[s][:, o // 128 : (o + sz) // 128, :],
                shard[s * RSUB : (s + 1) * RSUB, :],
                idx_sb[:, s * S + o // 16 : s * S + (o + sz) // 16],
                sz,
                size_regs[sz],
                VEC,
                queue_num=(s * nch + j) % 4,
            ).then_inc(sem_g[s * nch + j], 16)
    # Grouped writeouts (half a bin each, ~12-16KB per partition-descriptor
    # for near-peak HWDGE rate), alternating between the two HWDGE rings
    # (Sync -> qSPDynamicHW, Scalar -> qActDynamicHW) so writes overlap
    # gathers instead of serializing after them.
    # Two fat write groups per bin: small write packets disrupt the
    # latency-bound gather drain (measured +10us), so keep descriptors big
    # (>=12KB per partition) and only bias the split slightly early.
    groups = []  # (s, first_chunk_j, last_chunk_j)
    for s in range(NSUB):
        split = max(1, nch // 3)
        groups.append((s, 0, split - 1))
        groups.append((s, split, nch - 1))
    # issue in completion order of each group's last chunk (chunk k runs on
    # queue k%4 at depth k//4, so completion order ~ k)
    groups.sort(key=lambda g: g[0] * nch + g[2])
    n_writes = 0
    for gi, (s, j0, j1) in enumerate(groups):
        eng = nc.sync if gi % 2 == 0 else nc.scalar
        for j in range(j0, j1 + 1):
            eng.wait_ge(sem_g[s * nch + j], 16)
        o0 = chunks[j0][0]
        o1 = chunks[j1][0] + chunks[j1][1]
        eng.dma_start(
            out=out[:, (s * C + o0 // 128) * VEC : (s * C + o1 // 128) * VEC],
            in_=g_bufs[s][:, o0 // 128 : o1 // 128, :].rearrange("p c v -> p (c v)"),
        ).then_inc(sem_out, 16)
        n_writes += 1
    nc.sync.wait_ge(sem_out, 16 * n_writes)
    nc.finalize()
    return nc


def _gather_on_device(table: np.ndarray, v: np.ndarray) -> np.ndarray:
    """emb[i] = table[v[i]] computed on 8 NeuronCores."""
    global LAST_RUN
    from concourse.bass_utils import run_bass_kernel_spmd

    total = v.shape[0]
    nbins = NCORES * NSUB
    bin_id = (v // RSUB).astype(np.int32)
    local = (v - bin_id.astype(np.int64) * RSUB).astype(np.int16)

    # Sort by full index value: bins stay contiguous, and within each bin the
    # gather's 512B random reads walk HBM monotonically (page locality).
    order = np.argsort(v, kind="stable")
    counts = np.bincount(bin_id, minlength=nbins)
    assert counts.sum() == total
    N = max(P, ((int(counts.max()) + P - 1) // P) * P)
    S = N // 16
    C = N // 128

    ar = np.arange(N)
    wrap_r, wrap_c = ar % 16, ar // 16

    in_maps = []
    positions = []  # positions[c][s] = original indices of that bin's lookups
    bin_start = np.concatenate(([0], np.cumsum(counts)))
    for c in range(NCORES):
        idx_cols = []
        pos_c = []
        for s in range(NSUB):
            b = c * NSUB + s
            pos = order[bin_start[b] : bin_start[b + 1]]
            pos_c.append(pos)
            # Pad with index 0 (a valid row): num_idxs_reg must equal the
            # count of non-negative indices, and it is a compile-time
            # constant shared by all cores.
            li = np.zeros(N, np.int16)
            li[: len(pos)] = local[pos]
            wrapped = np.zeros((16, S), np.int16)
            wrapped[wrap_r, wrap_c] = li
            idx_cols.append(np.tile(wrapped, (8, 1)))
        in_maps.append(
            {
                "shard": np.ascontiguousarray(table[c * SHARD : (c + 1) * SHARD]),
                "idx": np.ascontiguousarray(np.concatenate(idx_cols, axis=1)),
            }
        )
        positions.append(pos_c)

    nc = _build_program(N)
    LAST_RUN = run_bass_kernel_spmd(nc, in_maps, list(range(NCORES)))
    res = LAST_RUN.results

    emb = np.empty((total, VEC), np.float32)
    for c in range(NCORES):
        o = np.asarray(res[c]["out"], dtype=np.float32).reshape(P, NSUB, C, VEC)
        for s in range(NSUB):
            rows = o[:, s].transpose(1, 0, 2).reshape(N, VEC)
            pos = positions[c][s]
            emb[pos] = rows[: len(pos)]
    return emb


def kernel(table, row_offsets, value_tensors, nnz_array=None, output_shape=None):
    table = np.ascontiguousarray(np.asarray(table, dtype=np.float32))
    assert table.shape == (VOCAB, VEC)
    v = np.asarray(value_tensors).astype(np.int64).ravel()
    total = v.shape[0]

    emb = _gather_on_device(table, v)

    n_rows = BATCH * SLOTS
    ro = np.asarray(row_offsets).astype(np.int64).ravel()
    if total == n_rows and np.array_equal(ro, np.arange(total + 1)):
        return emb.reshape(BATCH, SLOTS, VEC)
    # General CSR fallback (never hit with the reference's arange offsets):
    # sum-combine values per segment on the host.
    seg = np.searchsorted(ro, np.arange(total), side="right") - 1
    combined = np.zeros((n_rows, VEC), np.float32)
    np.add.at(combined, seg, emb)
    return combined.reshape(BATCH, SLOTS, VEC)


# revision 24
# speedup vs baseline: 1.1413x; 1.1413x over previous
"""Embedding lookup (gather) kernel for Trainium2, 8 NeuronCores.

Problem: out[i] = table[value_tensors[i]] for 212992 indices into a
[1M, 128] f32 table, reshaped to [8192, 26, 128]. (row_offsets is
arange, so the CSR segment-sum is the identity; a host-side fallback
handles the general case.)

Sharding: model-parallel by table row (range partition). The table is
split into 32 range bins of 31250 rows; core c owns bins 4c..4c+3
(125000 rows = 64MB per core). The host routes each lookup index to its
owning bin, each core gathers its rows on-device with the SWDGE
dma_gather instruction (one per bin; int16 local indices < 31250), and
the host scatters the gathered rows back to the original positions
(the "all-to-all" step of HugeCTR's localized embedding, done at
unshard time).

dma_gather layout (probed on HW): indices are int16, wrapped over 16
partitions (element i at [i % 16, i // 16]) and replicated to all 8
Q7-core partition groups; gathered row i lands at dst[i % 128, i // 128].
"""

import math

import numpy as np

VOCAB = 1_000_000
BATCH = 8192
SLOTS = 26
VEC = 128
NCORES = 8
NSUB = 4  # sub-shards (bins) per core; int16 gather idx needs rows <= 32767
RSUB = VOCAB // (NCORES * NSUB)  # 31250 rows per bin
SHARD = RSUB * NSUB  # 125000 rows per core
P = 128

LAST_RUN = None  # BassKernelResults of the most recent device run (for test.py)


def _build_program(N: int):
    """One SPMD program for all 8 cores. N = padded lookups per bin
    (multiple of 128; identical across cores/bins so num_idxs is a
    compile-time constant).

    Per core:
      shard [SHARD, VEC] f32      - this core's 4 bins, concatenated
      idx   [P, NSUB*S] int16     - wrapped local indices, S = N//16
      out   [P, NSUB*C*VEC] f32   - gathered rows, C = N//128
    """
    import concourse.bacc as bacc
    from concourse import mybir
    from concourse.library_config import mlp

    S = N // 16
    C = N // 128
    # Idxs per dma_gather: 768 -> 48 data descs + 1 sem desc per engine ring,
    # safely under the 64-descriptor packet ceiling (1024 -> 65 descs, which
    # is at/over the limit and produced rare device lockups).
    CH = 896

    chunks = []  # (start, size) within a bin, multiples of 128
    o = 0
    while o < N:
        chunks.append((o, min(CH, N - o)))
        o += CH
    nch = len(chunks)

    nc = bacc.Bacc("TRN2", num_swdge_queues=4)
    shard = nc.declare_dram_parameter(
        "shard", [SHARD, VEC], mybir.dt.float32, isOutput=False
    )
    idx = nc.declare_dram_parameter(
        "idx", [P, NSUB * S], mybir.dt.int16, isOutput=False
    )
    out = nc.declare_dram_parameter(
        "out", [P, NSUB * C * VEC], mybir.dt.float32, isOutput=True
    )

    sem_idx = nc.alloc_semaphore()
    sem_g = [
        nc.alloc_semaphore(f"sem_g{s}_{j}") for s in range(NSUB) for j in range(nch)
    ]
    sem_out = nc.alloc_semaphore()

    idx_sb = nc.alloc_sbuf_tensor("idx_sb", [P, NSUB * S], mybir.dt.int16).ap()
    g_bufs = [
        nc.alloc_sbuf_tensor(f"g{s}", [P, C, VEC], mybir.dt.float32).ap()
        for s in range(NSUB)
    ]

    nc.gpsimd.load_library(mlp)
    nc.sync.dma_start(out=idx_sb[:], in_=idx[:, :]).then_inc(sem_idx, 16)
    nc.gpsimd.wait_ge(sem_idx, 16)
    # Hoist num_idxs registers: one MOVE per distinct chunk size instead of
    # one per gather (each MOVE costs ~420ns of Pool sequencer time).
    size_regs = {sz: nc.gpsimd.to_reg(sz) for sz in sorted({sz for _, sz in chunks})}
    for s in range(NSUB):
        for j, (o, sz) in enumerate(chunks):
            # Round-robin over SWDGE queues: each queue_num runs on its own
            # Q7 core pair, parallelizing descriptor generation 4x.
            nc.gpsimd.dma_gather(
                g_bufs[s][:, o // 128 : (o + sz) // 128, :],
                shard[s * RSUB : (s + 1) * RSUB, :],
                idx_sb[:, s * S + o // 16 : s * S + (o + sz) // 16],
                sz,
                size_regs[sz],
                VEC,
                queue_num=(s * nch + j) % 4,
            ).then_inc(sem_g[s * nch + j], 16)
    # Grouped writeouts (half a bin each, ~12-16KB per partition-descriptor
    # for near-peak HWDGE rate), alternating between the two HWDGE rings
    # (Sync -> qSPDynamicHW, Scalar -> qActDynamicHW) so writes overlap
    # gathers instead of serializing after them.
    # Two fat write groups per bin (midpoint split): small write packets
    # disrupt the latency-bound gather drain (+10us measured), and a late
    # split grows the post-gather write tail (+13us measured); halves with
    # >=12KB partition descriptors measured best.
    groups = []  # (s, first_chunk_j, last_chunk_j)
    for s in range(NSUB):
        half = max(1, nch // 2)
        groups.append((s, 0, half - 1))
        groups.append((s, half, nch - 1))
    # issue in completion order of each group's last chunk (chunk k runs on
    # queue k%4 at depth k//4, so completion order ~ k)
    groups.sort(key=lambda g: g[0] * nch + g[2])
    n_writes = 0
    for gi, (s, j0, j1) in enumerate(groups):
        eng = nc.sync if gi % 2 == 0 else nc.scalar
        for j in range(j0, j1 + 1):
            eng.wait_ge(sem_g[s * nch + j], 16)
        o0 = chunks[j0][0]
        o1 = chunks[j1][0] + chunks[j1][1]
        eng.dma_start(
            out=out[:, (s * C + o0 // 128) * VEC : (s * C + o1 // 128) * VEC],
            in_=g_bufs[s][:, o0 // 128 : o1 // 128, :].rearrange("p c v -> p (c v)"),
        ).then_inc(sem_out, 16)
        n_writes += 1
    nc.sync.wait_ge(sem_out, 16 * n_writes)
    nc.finalize()
    return nc


def _gather_on_device(table: np.ndarray, v: np.ndarray) -> np.ndarray:
    """emb[i] = table[v[i]] computed on 8 NeuronCores."""
    global LAST_RUN
    from concourse.bass_utils import run_bass_kernel_spmd

    total = v.shape[0]
    nbins = NCORES * NSUB
    bin_id = (v // RSUB).astype(np.int32)
    local = (v - bin_id.astype(np.int64) * RSUB).astype(np.int16)

    # Sort by full index value: bins stay contiguous, and within each bin the
    # gather's 512B random reads walk HBM monotonically (page locality).
    order = np.argsort(v, kind="stable")
    counts = np.bincount(bin_id, minlength=nbins)
    assert counts.sum() == total
    N = max(P, ((int(counts.max()) + P - 1) // P) * P)
    S = N // 16
    C = N // 128

    ar = np.arange(N)
    wrap_r, wrap_c = ar % 16, ar // 16

    in_maps = []
    positions = []  # positions[c][s] = original indices of that bin's lookups
    bin_start = np.concatenate(([0], np.cumsum(counts)))
    for c in range(NCORES):
        idx_cols = []
        pos_c = []
        for s in range(NSUB):
            b = c * NSUB + s
            pos = order[bin_start[b] : bin_start[b + 1]]
            pos_c.append(pos)
            # Pad with index 0 (a valid row): num_idxs_reg must equal the
            # count of non-negative indices, and it is a compile-time
            # constant shared by all cores.
            li = np.zeros(N, np.int16)
            li[: len(pos)] = local[pos]
            wrapped = np.zeros((16, S), np.int16)
            wrapped[wrap_r, wrap_c] = li
            idx_cols.append(np.tile(wrapped, (8, 1)))
        in_maps.append(
            {
                "shard": np.ascontiguousarray(table[c * SHARD : (c + 1) * SHARD]),
                "idx": np.ascontiguousarray(np.concatenate(idx_cols, axis=1)),
            }
        )
        positions.append(pos_c)

    nc = _build_program(N)
    LAST_RUN = run_bass_kernel_spmd(nc, in_maps, list(range(NCORES)))
    res = LAST_RUN.results

    emb = np.empty((total, VEC), np.float32)
    for c in range(NCORES):
        o = np.asarray(res[c]["out"], dtype=np.float32).reshape(P, NSUB, C, VEC)
        for s in range(NSUB):
            rows = o[:, s].transpose(1, 0, 2).reshape(N, VEC)
            pos = positions[c][s]
            emb[pos] = rows[: len(pos)]
    return emb


def kernel(table, row_offsets, value_tensors, nnz_array=None, output_shape=None):
    table = np.ascontiguousarray(np.asarray(table, dtype=np.float32))
    assert table.shape == (VOCAB, VEC)
    v = np.asarray(value_tensors).astype(np.int64).ravel()
    total = v.shape[0]

    emb = _gather_on_device(table, v)

    n_rows = BATCH * SLOTS
    ro = np.asarray(row_offsets).astype(np.int64).ravel()
    if total == n_rows and np.array_equal(ro, np.arange(total + 1)):
        return emb.reshape(BATCH, SLOTS, VEC)
    # General CSR fallback (never hit with the reference's arange offsets):
    # sum-combine values per segment on the host.
    seg = np.searchsorted(ro, np.arange(total), side="right") - 1
    combined = np.zeros((n_rows, VEC), np.float32)
    np.add.at(combined, seg, emb)
    return combined.reshape(BATCH, SLOTS, VEC)


# revision 25
# speedup vs baseline: 1.1426x; 1.0011x over previous
"""Embedding lookup (gather) kernel for Trainium2, 8 NeuronCores.

Problem: out[i] = table[value_tensors[i]] for 212992 indices into a
[1M, 128] f32 table, reshaped to [8192, 26, 128]. (row_offsets is
arange, so the CSR segment-sum is the identity; a host-side fallback
handles the general case.)

Sharding: model-parallel by table row (range partition). The table is
split into 32 range bins of 31250 rows; core c owns bins 4c..4c+3
(125000 rows = 64MB per core). The host routes each lookup index to its
owning bin, each core gathers its rows on-device with the SWDGE
dma_gather instruction (one per bin; int16 local indices < 31250), and
the host scatters the gathered rows back to the original positions
(the "all-to-all" step of HugeCTR's localized embedding, done at
unshard time).

dma_gather layout (probed on HW): indices are int16, wrapped over 16
partitions (element i at [i % 16, i // 16]) and replicated to all 8
Q7-core partition groups; gathered row i lands at dst[i % 128, i // 128].
"""

import math

import numpy as np

VOCAB = 1_000_000
BATCH = 8192
SLOTS = 26
VEC = 128
NCORES = 8
NSUB = 4  # sub-shards (bins) per core; int16 gather idx needs rows <= 32767
RSUB = VOCAB // (NCORES * NSUB)  # 31250 rows per bin
SHARD = RSUB * NSUB  # 125000 rows per core
P = 128

LAST_RUN = None  # BassKernelResults of the most recent device run (for test.py)


def _build_program(N: int):
    """One SPMD program for all 8 cores. N = padded lookups per bin
    (multiple of 128; identical across cores/bins so num_idxs is a
    compile-time constant).

    Per core:
      shard [SHARD, VEC] f32      - this core's 4 bins, concatenated
      idx   [P, NSUB*S] int16     - wrapped local indices, S = N//16
      out   [P, NSUB*C*VEC] f32   - gathered rows, C = N//128
    """
    import concourse.bacc as bacc
    from concourse import mybir
    from concourse.library_config import mlp

    S = N // 16
    C = N // 128
    # Idxs per dma_gather: 896 -> 56 data descs + 1 sem desc per engine ring,
    # safely under the 64-descriptor packet ceiling (1024 -> 65 descs, which
    # is at/over the limit and produced rare device lockups).
    CH = 896

    chunks = []  # (start, size) within a bin, multiples of 128
    o = 0
    while o < N:
        chunks.append((o, min(CH, N - o)))
        o += CH
    nch = len(chunks)

    nc = bacc.Bacc("TRN2", num_swdge_queues=4)
    shard = nc.declare_dram_parameter(
        "shard", [SHARD, VEC], mybir.dt.float32, isOutput=False
    )
    idx = nc.declare_dram_parameter(
        "idx", [P, NSUB * S], mybir.dt.int16, isOutput=False
    )
    out = nc.declare_dram_parameter(
        "out", [P, NSUB * C * VEC], mybir.dt.float32, isOutput=True
    )

    sem_idx = nc.alloc_semaphore()
    sem_g = [
        nc.alloc_semaphore(f"sem_g{s}_{j}") for s in range(NSUB) for j in range(nch)
    ]
    sem_out = nc.alloc_semaphore()

    idx_sb = nc.alloc_sbuf_tensor("idx_sb", [P, NSUB * S], mybir.dt.int16).ap()
    g_bufs = [
        nc.alloc_sbuf_tensor(f"g{s}", [P, C, VEC], mybir.dt.float32).ap()
        for s in range(NSUB)
    ]

    nc.gpsimd.load_library(mlp)
    nc.sync.dma_start(out=idx_sb[:], in_=idx[:, :]).then_inc(sem_idx, 16)
    nc.gpsimd.wait_ge(sem_idx, 16)
    # Hoist num_idxs registers: one MOVE per distinct chunk size instead of
    # one per gather (each MOVE costs ~420ns of Pool sequencer time).
    size_regs = {sz: nc.gpsimd.to_reg(sz) for sz in sorted({sz for _, sz in chunks})}
    for s in range(NSUB):
        for j, (o, sz) in enumerate(chunks):
            # Round-robin over SWDGE queues: each queue_num runs on its own
            # Q7 core pair, parallelizing descriptor generation 4x.
            nc.gpsimd.dma_gather(
                g_bufs[s][:, o // 128 : (o + sz) // 128, :],
                shard[s * RSUB : (s + 1) * RSUB, :],
                idx_sb[:, s * S + o // 16 : s * S + (o + sz) // 16],
                sz,
                size_regs[sz],
                VEC,
                queue_num=(s * nch + j) % 4,
            ).then_inc(sem_g[s * nch + j], 16)
    # Grouped writeouts (half a bin each, ~12-16KB per partition-descriptor
    # for near-peak HWDGE rate), alternating between the two HWDGE rings
    # (Sync -> qSPDynamicHW, Scalar -> qActDynamicHW) so writes overlap
    # gathers instead of serializing after them.
    # Two fat write groups per bin (midpoint split): small write packets
    # disrupt the latency-bound gather drain (+10us measured), and a late
    # split grows the post-gather write tail (+13us measured); halves with
    # >=12KB partition descriptors measured best.
    groups = []  # (s, first_chunk_j, last_chunk_j)
    for s in range(NSUB):
        half = max(1, nch // 2)
        groups.append((s, 0, half - 1))
        groups.append((s, half, nch - 1))
    # issue in completion order of each group's last chunk (chunk k runs on
    # queue k%4 at depth k//4, so completion order ~ k)
    groups.sort(key=lambda g: g[0] * nch + g[2])
    n_writes = 0
    for gi, (s, j0, j1) in enumerate(groups):
        eng = nc.sync if gi % 2 == 0 else nc.scalar
        for j in range(j0, j1 + 1):
            eng.wait_ge(sem_g[s * nch + j], 16)
        o0 = chunks[j0][0]
        o1 = chunks[j1][0] + chunks[j1][1]
        eng.dma_start(
            out=out[:, (s * C + o0 // 128) * VEC : (s * C + o1 // 128) * VEC],
            in_=g_bufs[s][:, o0 // 128 : o1 // 128, :].rearrange("p c v -> p (c v)"),
        ).then_inc(sem_out, 16)
        n_writes += 1
    nc.sync.wait_ge(sem_out, 16 * n_writes)
    nc.finalize()
    return nc


def _gather_on_device(table: np.ndarray, v: np.ndarray) -> np.ndarray:
    """emb[i] = table[v[i]] computed on 8 NeuronCores."""
    global LAST_RUN
    from concourse.bass_utils import run_bass_kernel_spmd

    total = v.shape[0]
    nbins = NCORES * NSUB
    bin_id = (v // RSUB).astype(np.int32)
    local = (v - bin_id.astype(np.int64) * RSUB).astype(np.int16)

    # Sort by full index value: bins stay contiguous, and within each bin the
    # gather's 512B random reads walk HBM monotonically (page locality).
    order = np.argsort(v, kind="stable")
    counts = np.bincount(bin_id, minlength=nbins)
    assert counts.sum() == total
    N = max(P, ((int(counts.max()) + P - 1) // P) * P)
    S = N // 16
    C = N // 128

    ar = np.arange(N)
    wrap_r, wrap_c = ar % 16, ar // 16

    in_maps = []
    positions = []  # positions[c][s] = original indices of that bin's lookups
    bin_start = np.concatenate(([0], np.cumsum(counts)))
    for c in range(NCORES):
        idx_cols = []
        pos_c = []
        for s in range(NSUB):
            b = c * NSUB + s
            pos = order[bin_start[b] : bin_start[b + 1]]
            pos_c.append(pos)
            # Pad with index 0 (a valid row): num_idxs_reg must equal the
            # count of non-negative indices, and it is a compile-time
            # constant shared by all cores.
            li = np.zeros(N, np.int16)
            li[: len(pos)] = local[pos]
            wrapped = np.zeros((16, S), np.int16)
            wrapped[wrap_r, wrap_c] = li
            idx_cols.append(np.tile(wrapped, (8, 1)))
        in_maps.append(
            {
                "shard": np.ascontiguousarray(table[c * SHARD : (c + 1) * SHARD]),
                "idx": np.ascontiguousarray(np.concatenate(idx_cols, axis=1)),
            }
        )
        positions.append(pos_c)

    nc = _build_program(N)
    LAST_RUN = run_bass_kernel_spmd(nc, in_maps, list(range(NCORES)))
    res = LAST_RUN.results

    emb = np.empty((total, VEC), np.float32)
    for c in range(NCORES):
        o = np.asarray(res[c]["out"], dtype=np.float32).reshape(P, NSUB, C, VEC)
        for s in range(NSUB):
            rows = o[:, s].transpose(1, 0, 2).reshape(N, VEC)
            pos = positions[c][s]
            emb[pos] = rows[: len(pos)]
    return emb


def kernel(table, row_offsets, value_tensors, nnz_array=None, output_shape=None):
    table = np.ascontiguousarray(np.asarray(table, dtype=np.float32))
    assert table.shape == (VOCAB, VEC)
    v = np.asarray(value_tensors).astype(np.int64).ravel()
    total = v.shape[0]

    emb = _gather_on_device(table, v)

    n_rows = BATCH * SLOTS
    ro = np.asarray(row_offsets).astype(np.int64).ravel()
    if total == n_rows and np.array_equal(ro, np.arange(total + 1)):
        return emb.reshape(BATCH, SLOTS, VEC)
    # General CSR fallback (never hit with the reference's arange offsets):
    # sum-combine values per segment on the host.
    seg = np.searchsorted(ro, np.arange(total), side="right") - 1
    combined = np.zeros((n_rows, VEC), np.float32)
    np.add.at(combined, seg, emb)
    return combined.reshape(BATCH, SLOTS, VEC)
